# revision 16
# baseline (speedup 1.0000x reference)
"""Expert-parallel MoE (Kimi/DeepSeek-V3 style sparse block) on 8 trn2 NeuronCores.

Strategy:
  - Host computes the (tiny) sigmoid gate + group-limited top-2 routing in
    numpy float64 and gathers each expert's tokens into a fixed-capacity,
    pre-tiled bf16 batch (capacity 1184 = chunks of 512/512/160; max real
    per-expert load for these inputs is 1164).
  - Core e runs expert e's SwiGLU FFN over its gathered tokens, plus the
    shared-expert FFN (split into two F=512 halves) over token slice
    [512*e : 512*(e+1)].  All matmuls run on the PE in bf16 with fp32 PSUM
    accumulation.  PSUM banks are split 6 (up-proj) / 2 (down-proj) so
    consecutive chunks pipeline without bank conflicts.
  - All tensors are pre-tiled on host into SBUF layout ([128, K, N]) so
    every DMA is a maximal contiguous copy.
  - Host scatter-adds the weighted expert outputs and the shared outputs
    back into the full [B,S,D] result.
"""

from contextlib import ExitStack

import numpy as np
import ml_dtypes

import concourse.bacc as bacc
import concourse.tile as tile
import concourse.mybir as mybir
from concourse import bass_utils

# --- model dims (hardcoded per problem spec) ---
B, S, D = 2, 2048, 1024
T = B * S                 # 4096 tokens
E, F = 8, 512             # routed experts / expert intermediate
SH = 1024                 # shared intermediate
TOP_K, N_GROUP, TOPK_GROUP = 2, 4, 2
SCALE = 2.5

N_CORES = 8
P = 128                   # SBUF partitions
KD = D // P               # 8 contraction tiles for D
KF = F // P               # 4 contraction tiles for F
KS = SH // P              # 8 f-blocks for the shared intermediate
CHUNKS = [140, 512, 512]  # routed token chunks (sum = CAP); small first for fast start
CAP = sum(CHUNKS)         # per-expert token capacity
TSLICE = T // N_CORES     # 512 shared-expert tokens per core

F32 = mybir.dt.float32
BF16 = mybir.dt.bfloat16
NPBF16 = np.dtype(ml_dtypes.bfloat16)

_CACHE: dict = {}


def _emit(nc):
    """Per-core program: 3 expert-chunk FFN jobs + 2 shared-half FFN jobs.

    d-mapping is p-major (d = p*KD + k) so host-side tiling is a plain
    reshape; f/sh mappings are block-major (f = kf*128 + p), which is what
    the PSUM output partitioning naturally produces.
    """
    xe = nc.dram_tensor("xe", [P, KD * CAP], BF16, kind="ExternalInput").ap()
    xs = nc.dram_tensor("xs", [P, KD, TSLICE], BF16, kind="ExternalInput").ap()
    w1t = nc.dram_tensor("w1t", [P, KD, F], BF16, kind="ExternalInput").ap()
    w3t = nc.dram_tensor("w3t", [P, KD, F], BF16, kind="ExternalInput").ap()
    w2t = nc.dram_tensor("w2t", [P, KF, D], BF16, kind="ExternalInput").ap()
    sgt = nc.dram_tensor("sgt", [P, KD, SH], BF16, kind="ExternalInput").ap()
    sut = nc.dram_tensor("sut", [P, KD, SH], BF16, kind="ExternalInput").ap()
    sdt = nc.dram_tensor("sdt", [P, KS, D], BF16, kind="ExternalInput").ap()
    ye = nc.dram_tensor("ye", [D, CAP], BF16, kind="ExternalOutput").ap()
    ys = nc.dram_tensor("ys", [D, TSLICE], BF16, kind="ExternalOutput").ap()

    silu = mybir.ActivationFunctionType.Silu
    copy_fn = mybir.ActivationFunctionType.Copy

    with tile.TileContext(nc) as tc, ExitStack() as ctx:
        wpool = ctx.enter_context(tc.tile_pool(name="wpool", bufs=1))
        xpool = ctx.enter_context(tc.tile_pool(name="xpool", bufs=1))
        hpool = ctx.enter_context(tc.tile_pool(name="hpool", bufs=2))
        opool = ctx.enter_context(tc.tile_pool(name="opool", bufs=4))
        sopool = ctx.enter_context(tc.tile_pool(name="sopool", bufs=1))
        pspool = ctx.enter_context(tc.tile_pool(name="pspool", bufs=1, space="PSUM"))

        # Preload DMAs: split each tensor into k-slices spread across both
        # HWDGE issue queues (sync + scalar) and multiple DMA rings, and
        # interleave issues in need-order so the first expert chunk's operands
        # land within a couple of microseconds.  Tile dependency tracking is
        # range-granular, so the first matmuls only wait on their own slices.
        dma_engs = [nc.sync]
        _eng_i = [0]

        def dma(dst, src):
            eng = dma_engs[_eng_i[0] % len(dma_engs)]
            _eng_i[0] += 1
            eng.dma_start(dst, src)

        def alloc(pool, name, shape):
            return pool.tile(shape, BF16, name=name, tag=name)

        w1sb = alloc(wpool, "w1sb", [P, KD, F])
        w3sb = alloc(wpool, "w3sb", [P, KD, F])
        w2sb = alloc(wpool, "w2sb", [P, KF, D])
        xssb = alloc(xpool, "xssb", [P, KD, TSLICE])
        sgsb = alloc(wpool, "sgsb", [P, KD, SH])
        susb = alloc(wpool, "susb", [P, KD, SH])
        sdsb = alloc(wpool, "sdsb", [P, KS, D])
        xcsb = []
        xe_off = []
        off = 0
        for c, W in enumerate(CHUNKS):
            xcsb.append(alloc(xpool, f"xe{c}", [P, KD, W]))
            xe_off.append(off)
            off += KD * W

        def load_x_chunk(c, splits):
            W = CHUNKS[c]
            step = KD // splits
            for i in range(splits):
                a = i * step
                dma(
                    xcsb[c][:, a:a + step, :],
                    xe[:, xe_off[c] + a * W:xe_off[c] + (a + step) * W]
                    .rearrange("p (k w) -> p k w", k=step),
                )

        def load_w(sb, src, K, splits):
            step = K // splits
            for i in range(splits):
                a = i * step
                dma(sb[:, a:a + step, :], src[:, a:a + step, :])

        # need-order issue schedule
        load_w(w1sb, w1t, KD, 4)
        load_x_chunk(0, 2)
        load_w(w3sb, w3t, KD, 4)
        load_w(w2sb, w2t, KF, 2)
        load_x_chunk(1, 2)
        load_x_chunk(2, 2)
        load_w(xssb, xs, KD, 2)
        load_w(sgsb, sgt, KD, 2)
        load_w(susb, sut, KD, 2)
        load_w(sdsb, sdt, KS, 2)

        # fp32 accumulator for the two shared halves
        shared_sb = sopool.tile([P, KD, TSLICE], F32, name="shared_sb")

        # PSUM: 4 banks cycle through up-proj (h1/h3), 4 banks for down-proj
        up_tags = [f"u{i}" for i in range(4)]
        dn_tags = [f"d{i}" for i in range(4)]

        def ffn_job(xsb, x0, W, w1v, w3v, f0, w2v, kf0, mode, out_cols=None):
            """One FFN pass over W token columns.

            xsb[:, k, x0:x0+W] bf16 input; w1v/w3v [P, KD, *] stationary with
            f-offset f0; w2v [P, *, D] with k-tile offset kf0.
            """
            hts = []
            for mf in range(KF):
                h1 = pspool.tile([P, 512], F32, name="h1ps", tag=up_tags[(2 * mf) % 4])
                for k in range(KD):
                    nc.tensor.matmul(
                        h1[:, :W],
                        w1v[:, k, f0 + mf * P:f0 + (mf + 1) * P],
                        xsb[:, k, x0:x0 + W],
                        start=(k == 0), stop=(k == KD - 1),
                    )
                h3 = pspool.tile([P, 512], F32, name="h3ps", tag=up_tags[(2 * mf + 1) % 4])
                for k in range(KD):
                    nc.tensor.matmul(
                        h3[:, :W],
                        w3v[:, k, f0 + mf * P:f0 + (mf + 1) * P],
                        xsb[:, k, x0:x0 + W],
                        start=(k == 0), stop=(k == KD - 1),
                    )
                a = hpool.tile([P, 512], F32, name="asb", tag="silu")
                nc.scalar.activation(a[:, :W], h1[:, :W], silu)
                ht = hpool.tile([P, 512], BF16, name="htsb", tag=f"ht{mf}")
                nc.vector.tensor_mul(ht[:, :W], a[:, :W], h3[:, :W])
                hts.append(ht)

            for md in range(KD):
                yps = pspool.tile([P, 512], F32, name="yps", tag=dn_tags[md % 4])
                for kf in range(KF):
                    nc.tensor.matmul(
                        yps[:, :W],
                        w2v[:, kf0 + kf, md * P:(md + 1) * P],
                        hts[kf][:, :W],
                        start=(kf == 0), stop=(kf == KF - 1),
                    )
                if mode == "expert":
                    osb = opool.tile([P, 512], BF16, name="osb", tag="osb")
                    # alternate drain engine so neither scalar nor vector backs up
                    if md % 2 == 0:
                        nc.scalar.activation(osb[:, :W], yps[:, :W], copy_fn)
                    else:
                        nc.vector.tensor_copy(osb[:, :W], yps[:, :W])
                    nc.sync.dma_start(ye[md * P:(md + 1) * P, out_cols:out_cols + W], osb[:, :W])
                elif mode == "shared0":
                    if md % 2 == 0:
                        nc.scalar.activation(shared_sb[:, md, :], yps[:, :W], copy_fn)
                    else:
                        nc.vector.tensor_copy(shared_sb[:, md, :], yps[:, :W])
                else:  # shared1: accumulate into a bf16 out tile and emit
                    ysb = opool.tile([P, 512], BF16, name="ysb", tag="ysb")
                    nc.vector.tensor_add(ysb[:, :W], shared_sb[:, md, :], yps[:, :W])
                    # split into two DMAs to shorten the final drain
                    h = W // 2
                    nc.sync.dma_start(ys[md * P:(md + 1) * P, :h], ysb[:, :h])
                    nc.sync.dma_start(ys[md * P:(md + 1) * P, h:W], ysb[:, h:W])

        x0 = 0
        for c, W in enumerate(CHUNKS):
            ffn_job(xcsb[c], 0, W, w1sb, w3sb, 0, w2sb, 0, "expert", out_cols=x0)
            x0 += W
        ffn_job(xssb, 0, TSLICE, sgsb, susb, 0, sdsb, 0, "shared0")
        ffn_job(xssb, 0, TSLICE, sgsb, susb, F, sdsb, KF, "shared1")


def _get_nc():
    if "nc" not in _CACHE:
        nc = bacc.Bacc("TRN2", target_bir_lowering=False, debug=False,
                       num_devices=N_CORES)
        _emit(nc)
        nc.compile()
        _CACHE["nc"] = nc
    return _CACHE["nc"]


def _gate_numpy(x2d, gw, gb):
    """Replicates reference _moe_gate in float64 (routing-stable)."""
    xl = x2d.astype(np.float64)
    logits = xl @ gw.astype(np.float64).T
    scores = 1.0 / (1.0 + np.exp(-logits))
    sc = scores + gb.astype(np.float64)[None, :]
    grp = sc.reshape(T, N_GROUP, E // N_GROUP)
    group_scores = np.sort(grp, axis=-1)[:, :, -2:].sum(-1)
    gidx = np.argsort(-group_scores, axis=-1, kind="stable")[:, :TOPK_GROUP]
    gmask = np.zeros((T, N_GROUP), bool)
    gmask[np.arange(T)[:, None], gidx] = True
    smask = np.repeat(gmask, E // N_GROUP, axis=1)
    tmp = np.where(smask, sc, 0.0)
    tidx = np.argsort(-tmp, axis=-1, kind="stable")[:, :TOP_K]
    tw = np.take_along_axis(scores, tidx, axis=1)
    tw = tw / (tw.sum(-1, keepdims=True) + 1e-20)
    return tidx, (tw * SCALE).astype(np.float32)


def _ffn_host(x, w1e, w2e, w3e):
    """Host fallback for capacity-overflow tokens (rare)."""
    h = x @ w1e.T
    h = (h / (1.0 + np.exp(-h))) * (x @ w3e.T)
    return h @ w2e.T


def _tile_w_up(w):
    """w [F_out, D] -> stationary [P, KD, F_out]: (p,k,f) = w[f, p*KD+k]."""
    return np.ascontiguousarray(w.T, dtype=NPBF16).reshape(P, KD, w.shape[0])


def _tile_w_down(w):
    """w [D, F_in] -> stationary [P, F_in//P, D]: (p,kf,d) = w[d, kf*P+p]."""
    kf = w.shape[1] // P
    t = np.ascontiguousarray(w.T, dtype=NPBF16).reshape(kf, P, w.shape[0])
    return np.ascontiguousarray(t.transpose(1, 0, 2))


def kernel(hidden_states, gate_w, gate_bias, w1, w2, w3,
           shared_gate_w, shared_up_w, shared_down_w):
    hidden_states = np.asarray(hidden_states, np.float32)
    gate_w = np.asarray(gate_w, np.float32)
    gate_bias = np.asarray(gate_bias, np.float32)
    w1 = np.asarray(w1, np.float32)
    w2 = np.asarray(w2, np.float32)
    w3 = np.asarray(w3, np.float32)
    shared_gate_w = np.asarray(shared_gate_w, np.float32)
    shared_up_w = np.asarray(shared_up_w, np.float32)
    shared_down_w = np.asarray(shared_down_w, np.float32)

    x2d = hidden_states.reshape(T, D)
    tidx, tw = _gate_numpy(x2d, gate_w, gate_bias)

    # shared weights: identical on every core
    sgt = _tile_w_up(shared_gate_w)
    sut = _tile_w_up(shared_up_w)
    sdt = _tile_w_down(shared_down_w)

    x2dT_bf = np.ascontiguousarray(x2d.T, dtype=NPBF16)  # [D, T]
    xs_all = x2dT_bf.reshape(P, KD, T)

    in_maps = []
    idx_list, wt_list, n_list = [], [], []
    overflow = []
    for e in range(E):
        rows, slots = np.nonzero(tidx == e)
        n = len(rows)
        if n > CAP:
            overflow.append((e, rows[CAP:], slots[CAP:]))
            rows, slots = rows[:CAP], slots[:CAP]
            n = CAP
        idx_list.append(rows)
        wt_list.append(tw[rows, slots])
        n_list.append(n)
        xeT = np.zeros((D, CAP), NPBF16)
        xeT[:, :n] = x2dT_bf[:, rows]
        # pack chunk-major so each chunk's [P, KD, W] block is contiguous
        xe3 = xeT.reshape(P, KD, CAP)
        xe_packed = np.empty((P, KD * CAP), NPBF16)
        o = 0
        for W in CHUNKS:
            xe_packed[:, o:o + KD * W] = xe3[:, :, o // KD:o // KD + W].reshape(P, KD * W)
            o += KD * W
        in_maps.append({
            "xe": xe_packed,
            "xs": np.ascontiguousarray(xs_all[:, :, e * TSLICE:(e + 1) * TSLICE]),
            "w1t": _tile_w_up(w1[e]),
            "w3t": _tile_w_up(w3[e]),
            "w2t": _tile_w_down(w2[e]),
            "sgt": sgt,
            "sut": sut,
            "sdt": sdt,
        })

    nc = _get_nc()
    res = bass_utils.run_bass_kernel_spmd(
        nc, in_maps, core_ids=list(range(N_CORES))
    )
    _CACHE["last_res"] = res

    y = np.zeros((T, D), np.float32)
    for e in range(E):
        n = n_list[e]
        out = res.results[e]
        if n:
            yeo = out["ye"][:, :n].T.astype(np.float32)  # [n, D]
            y[idx_list[e]] += wt_list[e][:, None] * yeo
        sl = slice(e * TSLICE, (e + 1) * TSLICE)
        y[sl] += out["ys"].T.astype(np.float32)
    for e, rows, slots in overflow:
        y[rows] += tw[rows, slots][:, None] * _ffn_host(x2d[rows], w1[e], w2[e], w3[e])

    return y.reshape(B, S, D)


# revision 22
# speedup vs baseline: 1.0112x; 1.0112x over previous
"""Expert-parallel MoE (Kimi/DeepSeek-V3 style sparse block) on 8 trn2 NeuronCores.

Strategy:
  - Host computes the (tiny) sigmoid gate + group-limited top-2 routing in
    numpy float64 and gathers each expert's tokens into a fixed-capacity,
    pre-tiled bf16 batch (capacity 1184 = chunks of 512/512/160; max real
    per-expert load for these inputs is 1164).
  - Core e runs expert e's SwiGLU FFN over its gathered tokens, plus the
    shared-expert FFN (split into two F=512 halves) over token slice
    [512*e : 512*(e+1)].  All matmuls run on the PE in bf16 with fp32 PSUM
    accumulation.  PSUM banks are split 6 (up-proj) / 2 (down-proj) so
    consecutive chunks pipeline without bank conflicts.
  - All tensors are pre-tiled on host into SBUF layout ([128, K, N]) so
    every DMA is a maximal contiguous copy.
  - Host scatter-adds the weighted expert outputs and the shared outputs
    back into the full [B,S,D] result.
"""

from contextlib import ExitStack

import numpy as np
import ml_dtypes

import concourse.bacc as bacc
import concourse.tile as tile
import concourse.mybir as mybir
from concourse import bass_utils

# --- model dims (hardcoded per problem spec) ---
B, S, D = 2, 2048, 1024
T = B * S                 # 4096 tokens
E, F = 8, 512             # routed experts / expert intermediate
SH = 1024                 # shared intermediate
TOP_K, N_GROUP, TOPK_GROUP = 2, 4, 2
SCALE = 2.5

N_CORES = 8
P = 128                   # SBUF partitions
KD = D // P               # 8 contraction tiles for D
KF = F // P               # 4 contraction tiles for F
KS = SH // P              # 8 f-blocks for the shared intermediate
CHUNKS = [140, 512, 512]  # routed token chunks (sum = CAP); small first for fast start
CAP = sum(CHUNKS)         # per-expert token capacity
TSLICE = T // N_CORES     # 512 shared-expert tokens per core

F32 = mybir.dt.float32
BF16 = mybir.dt.bfloat16
NPBF16 = np.dtype(ml_dtypes.bfloat16)

_CACHE: dict = {}


def _emit(nc):
    """Per-core program: 3 expert-chunk FFN jobs + 2 shared-half FFN jobs.

    d-mapping is p-major (d = p*KD + k) so host-side tiling is a plain
    reshape; f/sh mappings are block-major (f = kf*128 + p), which is what
    the PSUM output partitioning naturally produces.
    """
    xe = nc.dram_tensor("xe", [P, KD * CAP], BF16, kind="ExternalInput").ap()
    xs = nc.dram_tensor("xs", [P, KD, TSLICE], BF16, kind="ExternalInput").ap()
    w1t = nc.dram_tensor("w1t", [P, KD, F], BF16, kind="ExternalInput").ap()
    w3t = nc.dram_tensor("w3t", [P, KD, F], BF16, kind="ExternalInput").ap()
    w2t = nc.dram_tensor("w2t", [P, KF, D], BF16, kind="ExternalInput").ap()
    sgt = nc.dram_tensor("sgt", [P, KD, SH], BF16, kind="ExternalInput").ap()
    sut = nc.dram_tensor("sut", [P, KD, SH], BF16, kind="ExternalInput").ap()
    sdt = nc.dram_tensor("sdt", [P, KS, D], BF16, kind="ExternalInput").ap()
    ye = nc.dram_tensor("ye", [D, CAP], BF16, kind="ExternalOutput").ap()
    ys = nc.dram_tensor("ys", [D, TSLICE], BF16, kind="ExternalOutput").ap()

    silu = mybir.ActivationFunctionType.Silu
    copy_fn = mybir.ActivationFunctionType.Copy

    with tile.TileContext(nc) as tc, ExitStack() as ctx:
        wpool = ctx.enter_context(tc.tile_pool(name="wpool", bufs=1))
        xpool = ctx.enter_context(tc.tile_pool(name="xpool", bufs=1))
        hpool = ctx.enter_context(tc.tile_pool(name="hpool", bufs=2))
        opool = ctx.enter_context(tc.tile_pool(name="opool", bufs=2))
        sopool = ctx.enter_context(tc.tile_pool(name="sopool", bufs=1))
        pspool = ctx.enter_context(tc.tile_pool(name="pspool", bufs=1, space="PSUM"))

        # Preload DMAs: split each tensor into k-slices spread across both
        # HWDGE issue queues (sync + scalar) and multiple DMA rings, and
        # interleave issues in need-order so the first expert chunk's operands
        # land within a couple of microseconds.  Tile dependency tracking is
        # range-granular, so the first matmuls only wait on their own slices.
        dma_engs = [nc.sync]
        _eng_i = [0]

        def dma(dst, src):
            eng = dma_engs[_eng_i[0] % len(dma_engs)]
            _eng_i[0] += 1
            eng.dma_start(dst, src)

        def alloc(pool, name, shape):
            return pool.tile(shape, BF16, name=name, tag=name)

        w1sb = alloc(wpool, "w1sb", [P, KD, F])
        w3sb = alloc(wpool, "w3sb", [P, KD, F])
        w2sb = alloc(wpool, "w2sb", [P, KF, D])
        xssb = alloc(xpool, "xssb", [P, KD, TSLICE])
        sgsb = alloc(wpool, "sgsb", [P, KD, SH])
        susb = alloc(wpool, "susb", [P, KD, SH])
        sdsb = alloc(wpool, "sdsb", [P, KS, D])
        xcsb = []
        xe_off = []
        off = 0
        for c, W in enumerate(CHUNKS):
            xcsb.append(alloc(xpool, f"xe{c}", [P, KD, W]))
            xe_off.append(off)
            off += KD * W

        def load_x_chunk(c, splits):
            W = CHUNKS[c]
            step = KD // splits
            for i in range(splits):
                a = i * step
                dma(
                    xcsb[c][:, a:a + step, :],
                    xe[:, xe_off[c] + a * W:xe_off[c] + (a + step) * W]
                    .rearrange("p (k w) -> p k w", k=step),
                )

        def load_w(sb, src, K, splits):
            step = K // splits
            for i in range(splits):
                a = i * step
                dma(sb[:, a:a + step, :], src[:, a:a + step, :])

        # need-order issue schedule; DMA issues cost ~0.6us each on the sync
        # queue, so use FEW issues ordered by first use.  Only the very first
        # operands (w1/xe0/w3) are halved so the first matmuls start sooner.
        H = KD // 2
        W0 = CHUNKS[0]
        dma(w1sb[:, :H, :], w1t[:, :H, :])
        dma(xcsb[0][:, :H, :],
            xe[:, :H * W0].rearrange("p (k w) -> p k w", k=H))
        dma(w3sb[:, :H, :], w3t[:, :H, :])
        dma(w1sb[:, H:, :], w1t[:, H:, :])
        dma(xcsb[0][:, H:, :],
            xe[:, H * W0:KD * W0].rearrange("p (k w) -> p k w", k=H))
        dma(w3sb[:, H:, :], w3t[:, H:, :])
        load_w(w2sb, w2t, KF, 1)
        load_x_chunk(1, 1)
        load_x_chunk(2, 1)
        load_w(xssb, xs, KD, 1)
        load_w(sgsb, sgt, KD, 1)
        load_w(susb, sut, KD, 1)
        load_w(sdsb, sdt, KS, 1)

        # fp32 accumulator for the two shared halves
        shared_sb = sopool.tile([P, KD, TSLICE], F32, name="shared_sb")

        # PSUM: 4 banks cycle through up-proj (h1/h3), 4 banks for down-proj
        up_tags = [f"u{i}" for i in range(4)]
        dn_tags = [f"d{i}" for i in range(4)]

        def ffn_job(xsb, x0, W, w1v, w3v, f0, w2v, kf0, mode, out_cols=None):
            """One FFN pass over W token columns.

            xsb[:, k, x0:x0+W] bf16 input; w1v/w3v [P, KD, *] stationary with
            f-offset f0; w2v [P, *, D] with k-tile offset kf0.
            """
            ojob = None
            if mode in ("expert", "shared1"):
                ojob = opool.tile([P, KD, 512], BF16, name="ojob", tag="ojob")
            hts = []
            for mf in range(KF):
                h1 = pspool.tile([P, 512], F32, name="h1ps", tag=up_tags[(2 * mf) % 4])
                for k in range(KD):
                    nc.tensor.matmul(
                        h1[:, :W],
                        w1v[:, k, f0 + mf * P:f0 + (mf + 1) * P],
                        xsb[:, k, x0:x0 + W],
                        start=(k == 0), stop=(k == KD - 1),
                    )
                h3 = pspool.tile([P, 512], F32, name="h3ps", tag=up_tags[(2 * mf + 1) % 4])
                for k in range(KD):
                    nc.tensor.matmul(
                        h3[:, :W],
                        w3v[:, k, f0 + mf * P:f0 + (mf + 1) * P],
                        xsb[:, k, x0:x0 + W],
                        start=(k == 0), stop=(k == KD - 1),
                    )
                a = hpool.tile([P, 512], F32, name="asb", tag="silu")
                nc.scalar.activation(a[:, :W], h1[:, :W], silu)
                ht = hpool.tile([P, 512], BF16, name="htsb", tag=f"ht{mf}")
                nc.vector.tensor_mul(ht[:, :W], a[:, :W], h3[:, :W])
                hts.append(ht)

            for md in range(KD):
                yps = pspool.tile([P, 512], F32, name="yps", tag=dn_tags[md % 4])
                for kf in range(KF):
                    nc.tensor.matmul(
                        yps[:, :W],
                        w2v[:, kf0 + kf, md * P:(md + 1) * P],
                        hts[kf][:, :W],
                        start=(kf == 0), stop=(kf == KF - 1),
                    )
                # alternate drain engine so neither scalar nor vector backs up
                if mode == "expert":
                    if md % 2 == 0:
                        nc.scalar.activation(ojob[:, md, :W], yps[:, :W], copy_fn)
                    else:
                        nc.vector.tensor_copy(ojob[:, md, :W], yps[:, :W])
                elif mode == "shared0":
                    if md % 2 == 0:
                        nc.scalar.activation(shared_sb[:, md, :], yps[:, :W], copy_fn)
                    else:
                        nc.vector.tensor_copy(shared_sb[:, md, :], yps[:, :W])
                else:  # shared1: accumulate into the bf16 out tile
                    nc.vector.tensor_add(ojob[:, md, :W], shared_sb[:, md, :], yps[:, :W])

            # one consolidated output DMA per job (issues are expensive)
            if mode == "expert":
                nc.sync.dma_start(
                    ye[:, out_cols:out_cols + W].rearrange("(m p) w -> p m w", p=P),
                    ojob[:, :, :W],
                )
            elif mode == "shared1":
                nc.sync.dma_start(
                    ys.rearrange("(m p) w -> p m w", p=P), ojob[:, :, :W]
                )

        x0 = 0
        for c, W in enumerate(CHUNKS):
            ffn_job(xcsb[c], 0, W, w1sb, w3sb, 0, w2sb, 0, "expert", out_cols=x0)
            x0 += W
        ffn_job(xssb, 0, TSLICE, sgsb, susb, 0, sdsb, 0, "shared0")
        ffn_job(xssb, 0, TSLICE, sgsb, susb, F, sdsb, KF, "shared1")


def _get_nc():
    if "nc" not in _CACHE:
        nc = bacc.Bacc("TRN2", target_bir_lowering=False, debug=False,
                       num_devices=N_CORES)
        _emit(nc)
        nc.compile()
        _CACHE["nc"] = nc
    return _CACHE["nc"]


def _gate_numpy(x2d, gw, gb):
    """Replicates reference _moe_gate in float64 (routing-stable)."""
    xl = x2d.astype(np.float64)
    logits = xl @ gw.astype(np.float64).T
    scores = 1.0 / (1.0 + np.exp(-logits))
    sc = scores + gb.astype(np.float64)[None, :]
    grp = sc.reshape(T, N_GROUP, E // N_GROUP)
    group_scores = np.sort(grp, axis=-1)[:, :, -2:].sum(-1)
    gidx = np.argsort(-group_scores, axis=-1, kind="stable")[:, :TOPK_GROUP]
    gmask = np.zeros((T, N_GROUP), bool)
    gmask[np.arange(T)[:, None], gidx] = True
    smask = np.repeat(gmask, E // N_GROUP, axis=1)
    tmp = np.where(smask, sc, 0.0)
    tidx = np.argsort(-tmp, axis=-1, kind="stable")[:, :TOP_K]
    tw = np.take_along_axis(scores, tidx, axis=1)
    tw = tw / (tw.sum(-1, keepdims=True) + 1e-20)
    return tidx, (tw * SCALE).astype(np.float32)


def _ffn_host(x, w1e, w2e, w3e):
    """Host fallback for capacity-overflow tokens (rare)."""
    h = x @ w1e.T
    h = (h / (1.0 + np.exp(-h))) * (x @ w3e.T)
    return h @ w2e.T


def _tile_w_up(w):
    """w [F_out, D] -> stationary [P, KD, F_out]: (p,k,f) = w[f, p*KD+k]."""
    return np.ascontiguousarray(w.T, dtype=NPBF16).reshape(P, KD, w.shape[0])


def _tile_w_down(w):
    """w [D, F_in] -> stationary [P, F_in//P, D]: (p,kf,d) = w[d, kf*P+p]."""
    kf = w.shape[1] // P
    t = np.ascontiguousarray(w.T, dtype=NPBF16).reshape(kf, P, w.shape[0])
    return np.ascontiguousarray(t.transpose(1, 0, 2))


def kernel(hidden_states, gate_w, gate_bias, w1, w2, w3,
           shared_gate_w, shared_up_w, shared_down_w):
    hidden_states = np.asarray(hidden_states, np.float32)
    gate_w = np.asarray(gate_w, np.float32)
    gate_bias = np.asarray(gate_bias, np.float32)
    w1 = np.asarray(w1, np.float32)
    w2 = np.asarray(w2, np.float32)
    w3 = np.asarray(w3, np.float32)
    shared_gate_w = np.asarray(shared_gate_w, np.float32)
    shared_up_w = np.asarray(shared_up_w, np.float32)
    shared_down_w = np.asarray(shared_down_w, np.float32)

    x2d = hidden_states.reshape(T, D)
    tidx, tw = _gate_numpy(x2d, gate_w, gate_bias)

    # shared weights: identical on every core
    sgt = _tile_w_up(shared_gate_w)
    sut = _tile_w_up(shared_up_w)
    sdt = _tile_w_down(shared_down_w)

    x2dT_bf = np.ascontiguousarray(x2d.T, dtype=NPBF16)  # [D, T]
    xs_all = x2dT_bf.reshape(P, KD, T)

    in_maps = []
    idx_list, wt_list, n_list = [], [], []
    overflow = []
    for e in range(E):
        rows, slots = np.nonzero(tidx == e)
        n = len(rows)
        if n > CAP:
            overflow.append((e, rows[CAP:], slots[CAP:]))
            rows, slots = rows[:CAP], slots[:CAP]
            n = CAP
        idx_list.append(rows)
        wt_list.append(tw[rows, slots])
        n_list.append(n)
        xeT = np.zeros((D, CAP), NPBF16)
        xeT[:, :n] = x2dT_bf[:, rows]
        # pack chunk-major so each chunk's [P, KD, W] block is contiguous
        xe3 = xeT.reshape(P, KD, CAP)
        xe_packed = np.empty((P, KD * CAP), NPBF16)
        o = 0
        for W in CHUNKS:
            xe_packed[:, o:o + KD * W] = xe3[:, :, o // KD:o // KD + W].reshape(P, KD * W)
            o += KD * W
        in_maps.append({
            "xe": xe_packed,
            "xs": np.ascontiguousarray(xs_all[:, :, e * TSLICE:(e + 1) * TSLICE]),
            "w1t": _tile_w_up(w1[e]),
            "w3t": _tile_w_up(w3[e]),
            "w2t": _tile_w_down(w2[e]),
            "sgt": sgt,
            "sut": sut,
            "sdt": sdt,
        })

    nc = _get_nc()
    res = bass_utils.run_bass_kernel_spmd(
        nc, in_maps, core_ids=list(range(N_CORES))
    )
    _CACHE["last_res"] = res

    y = np.zeros((T, D), np.float32)
    for e in range(E):
        n = n_list[e]
        out = res.results[e]
        if n:
            yeo = out["ye"][:, :n].T.astype(np.float32)  # [n, D]
            y[idx_list[e]] += wt_list[e][:, None] * yeo
        sl = slice(e * TSLICE, (e + 1) * TSLICE)
        y[sl] += out["ys"].T.astype(np.float32)
    for e, rows, slots in overflow:
        y[rows] += tw[rows, slots][:, None] * _ffn_host(x2d[rows], w1[e], w2[e], w3[e])

    return y.reshape(B, S, D)
